# revision 8
# baseline (speedup 1.0000x reference)
"""Adaptive-input softmax (AdaptiveLogSoftmaxWithLoss 'softmax' mode) on 8 TRN2 NeuronCores.

Problem: x [2,1024,512] f32 -> out [2,1024,100000] f32.
  head softmax over 20002 logits (20000 head tokens + 2 tail-cluster logits),
  tail_i softmax over its vocab, scaled by its cluster probability.

Strategy (vocab-parallel over 8 cores):
  Each core owns 1/8 of each softmax group: 2500 head cols + 3750 tail0 cols +
  6250 tail1 cols = a [2048 tokens, 12500] f32 output shard (102 MB — the
  kernel is memory-bound on this write; per-core HBM ~358 GB/s -> ~290 us).
  Per 128-token tile: matmul logits (bf16 inputs, f32 PSUM, [128,2048] PSUM
  tiles), exp on ScalarE into a bf16 SBUF tile (accumulating per-group partial
  sums), AllGather the per-group partial sums across cores every 2 tiles
  (tiny, ~5 us, overlapped) and reduce locally, normalize on VectorE (4x bf16
  mode), DMA out with bf16->f32 cast (SWDGE). The 2 cluster logits are
  computed redundantly on every core (identical values) and added to the
  gathered head-slice sums locally.

Host side: shard/transpose/cast inputs (bf16), reassemble output shards.
"""

import numpy as np
import ml_dtypes
from contextlib import ExitStack

import concourse.bass as bass
import concourse.mybir as mybir
import concourse.tile as tile
from concourse import bacc
from concourse.bass import ts
from concourse.bass_utils import run_bass_kernel_spmd

NCORES = 8
H = 512
TOK = 2048           # 2*1024 tokens
PT = 128             # tokens per tile (partition dim)
NTILE = TOK // PT    # 16
HEAD = 2500          # head vocab shard per core (20000/8)
T0 = 3750            # tail0 shard (30000/8)
T1 = 6250            # tail1 shard (50000/8)
OUT_COLS = HEAD + T0 + T1   # 12500
P0 = 128             # tail0 projection dim
P1 = 32              # tail1 projection dim
BF16 = mybir.dt.bfloat16
F32 = mybir.dt.float32

LCOLS = 2502 + T0 + T1      # 12502 logical logit cols (head+cl | t0 | t1)
PSW = 2048                  # psum tile width (4 banks)
NPT = (LCOLS + PSW - 1) // PSW   # 7 psum tiles per token tile

# logical col segments: which weight/contract path computes a col
SEG_A = (0, 2502)           # head + cluster, K=512 via xT
SEG_B = (2502, 2502 + T0)   # tail0, K=128 via p0T
SEG_C = (2502 + T0, LCOLS)  # tail1, K=32  via p1T
# groups for softmax sums / output placement
G_HEAD = (0, 2500)
G_CL = (2500, 2502)
G_T0 = SEG_B
G_T1 = SEG_C

EXP = mybir.ActivationFunctionType.Exp
ADD = mybir.AluOpType.add
AXX = mybir.AxisListType.X


def _spans():
    """Per psum-tile matmul spans (bank x segment) and act spans (group)."""
    tiles = []
    slot = {"head": [], "t0": [], "t1": []}
    nslots = 0
    for t in range(NPT):
        t0c, t1c = t * PSW, min((t + 1) * PSW, LCOLS)
        mms = []
        for b in range(t0c, t1c, 512):
            be = min(b + 512, t1c)
            for (seg, (s0, s1)) in (("A", SEG_A), ("B", SEG_B), ("C", SEG_C)):
                g0, g1 = max(b, s0), min(be, s1)
                if g0 < g1:
                    mms.append((seg, g0, g1))
        acts = []
        for (grp, (s0, s1)) in (("head", G_HEAD), ("cl", G_CL),
                                ("t0", G_T0), ("t1", G_T1)):
            g0, g1 = max(t0c, s0), min(t1c, s1)
            if g0 < g1:
                if grp == "cl":
                    acts.append((grp, g0, g1, None))
                else:
                    acts.append((grp, g0, g1, nslots))
                    slot[grp].append(nslots)
                    nslots += 1
        tiles.append((t0c, mms, acts))
    return tiles, slot, nslots


PT_SPANS, SLOTS, NACC = _spans()
assert NACC == 9, NACC
# slot ranges must be contiguous per group for the reduce
assert SLOTS["head"] == [0, 1] and SLOTS["t0"] == [2, 3, 4] \
    and SLOTS["t1"] == [5, 6, 7, 8]

GRP_TILES = 2               # token tiles batched per AllGather (default)


def build_nc(repeats: int = 1, et_bufs: int = 5, et_f32: bool = False,
             grp_tiles: int = GRP_TILES, split_out: int = 1) -> bass.Bass:
    nc = bacc.Bacc("TRN2", target_bir_lowering=False, debug=False,
                   num_devices=NCORES)
    xt_d = nc.declare_dram_parameter("xt", [H, TOK], BF16, isOutput=False)
    hw_d = nc.declare_dram_parameter("hw", [H, HEAD + 2], BF16, isOutput=False)
    tp0_d = nc.declare_dram_parameter("tp0", [H, P0], BF16, isOutput=False)
    tw0_d = nc.declare_dram_parameter("tw0", [P0, T0], BF16, isOutput=False)
    tp1_d = nc.declare_dram_parameter("tp1", [H, P1], BF16, isOutput=False)
    tw1_d = nc.declare_dram_parameter("tw1", [P1, T1], BF16, isOutput=False)
    out_d = nc.declare_dram_parameter("out", [TOK, OUT_COLS], F32, isOutput=True)

    et_dt = F32 if et_f32 else BF16

    with tile.TileContext(nc) as tc, ExitStack() as ctx:
        singles = ctx.enter_context(tc.tile_pool(name="singles", bufs=1))
        psum = ctx.enter_context(tc.tile_pool(name="psum", bufs=2, space="PSUM"))
        etp = ctx.enter_context(tc.tile_pool(name="etp", bufs=et_bufs))
        small = ctx.enter_context(tc.tile_pool(name="small", bufs=4))
        dram = ctx.enter_context(tc.tile_pool(name="dram", bufs=4, space="DRAM"))

        # ---- stage weights + xT in SBUF (bf16) ----
        xt_sb = singles.tile([PT, 4, TOK], BF16, name="xt_sb")
        hw_sb = singles.tile([PT, 4, HEAD + 2], BF16, name="hw_sb")
        tp0_sb = singles.tile([PT, 4, P0], BF16, name="tp0_sb")
        tp1_sb = singles.tile([PT, 4, P1], BF16, name="tp1_sb")
        tw0_sb = singles.tile([P0, T0], BF16, name="tw0_sb")
        tw1_sb = singles.tile([P1, T1], BF16, name="tw1_sb")
        for s in range(4):
            nc.sync.dma_start(out=xt_sb[:, s, :], in_=xt_d[ts(s, PT), :])
            nc.sync.dma_start(out=hw_sb[:, s, :], in_=hw_d[ts(s, PT), :])
            nc.sync.dma_start(out=tp0_sb[:, s, :], in_=tp0_d[ts(s, PT), :])
            nc.sync.dma_start(out=tp1_sb[:, s, :], in_=tp1_d[ts(s, PT), :])
        nc.sync.dma_start(out=tw0_sb[:, :], in_=tw0_d[:, :])
        nc.sync.dma_start(out=tw1_sb[:, :], in_=tw1_d[:, :])

        # ---- low-rank projections, transposed: p0T [128, 2048], p1T [32, 2048]
        p0t_sb = singles.tile([P0, TOK], BF16, name="p0t_sb")
        p1t_sb = singles.tile([P1, TOK], BF16, name="p1t_sb")
        ps0 = psum.tile([PT, PSW], F32, name="ps0", tag="ps")
        ps1 = psum.tile([PT, PSW], F32, name="ps1", tag="ps")
        for nb in range(TOK // 512):
            for k in range(4):
                nc.tensor.matmul(ps0[:, ts(nb, 512)], tp0_sb[:, k, :],
                                 xt_sb[:, k, ts(nb, 512)],
                                 start=(k == 0), stop=(k == 3))
            for k in range(4):
                nc.tensor.matmul(ps1[:P1, ts(nb, 512)], tp1_sb[:, k, :],
                                 xt_sb[:, k, ts(nb, 512)],
                                 start=(k == 0), stop=(k == 3))
        nc.vector.tensor_copy(p0t_sb[:, :], ps0[:, :])
        nc.vector.tensor_copy(p1t_sb[:, :], ps1[:P1, :])

        # ---- main loop ----
        ngrp = NTILE // grp_tiles
        pgw = 4 * grp_tiles
        for r in range(repeats):
            for g in range(ngrp):
                ets, cls = [], []
                pg = small.tile([PT, pgw], F32, name="pg", tag="pg")
                for jj in range(grp_tiles):
                    j = g * grp_tiles + jj
                    et = etp.tile([PT, OUT_COLS], et_dt, name="et", tag="et")
                    cl = small.tile([PT, 2], F32, name="cl", tag=f"cl{jj}")
                    sacc = small.tile([PT, NACC], F32, name="sacc", tag=f"sacc{jj}")
                    ets.append(et)
                    cls.append(cl)
                    for (base, mms, acts) in PT_SPANS:
                        pt = psum.tile([PT, PSW], F32, name="pt", tag="ps")
                        for (seg, g0, g1) in mms:
                            o = g0 - base
                            n = g1 - g0
                            if seg == "A":
                                for k in range(4):
                                    nc.tensor.matmul(
                                        pt[:, o:o + n], xt_sb[:, k, ts(j, PT)],
                                        hw_sb[:, k, g0:g1],
                                        start=(k == 0), stop=(k == 3))
                            elif seg == "B":
                                nc.tensor.matmul(
                                    pt[:, o:o + n], p0t_sb[:, ts(j, PT)],
                                    tw0_sb[:, g0 - SEG_B[0]:g1 - SEG_B[0]])
                            else:
                                nc.tensor.matmul(
                                    pt[:, o:o + n], p1t_sb[:, ts(j, PT)],
                                    tw1_sb[:, g0 - SEG_C[0]:g1 - SEG_C[0]])
                        for (grp, g0, g1, sl) in acts:
                            o = g0 - base
                            n = g1 - g0
                            if grp == "cl":
                                nc.scalar.activation(cl[:, :], pt[:, o:o + n], EXP)
                                continue
                            # output col: head at g0, t0/t1 shifted by the 2 cl cols
                            oc = g0 if grp == "head" else g0 - 2
                            nc.scalar.activation(et[:, oc:oc + n], pt[:, o:o + n],
                                                 EXP, accum_out=sacc[:, sl:sl + 1])
                    # per-group partial sums -> pg cols [4*jj .. 4*jj+3)
                    c = 4 * jj
                    nc.vector.tensor_reduce(pg[:, c:c + 1], sacc[:, 0:2], AXX, ADD)
                    nc.vector.tensor_reduce(pg[:, c + 1:c + 2], sacc[:, 2:5], AXX, ADD)
                    nc.vector.tensor_reduce(pg[:, c + 2:c + 3], sacc[:, 5:9], AXX, ADD)
                    nc.vector.tensor_reduce(pg[:, c + 3:c + 4], sacc[:, 8:9], AXX, ADD)

                # cross-core AllGather of the partial sums, local reduce
                ccin = dram.tile([PT, pgw], F32, name="ccin", tag="ccin")
                ccout = dram.tile([NCORES * PT, pgw], F32, name="ccout", tag="ccout",
                                  addr_space="Shared")
                nc.sync.dma_start(out=ccin[:, :], in_=pg[:, :])
                nc.gpsimd.collective_compute(
                    "AllGather", mybir.AluOpType.bypass,
                    replica_groups=[list(range(NCORES))],
                    ins=[ccin[:, :].opt()], outs=[ccout[:, :].opt()])
                agb = small.tile([PT, NCORES, pgw], F32, name="agb", tag="agb")
                nc.sync.dma_start(
                    out=agb[:, :, :],
                    in_=ccout.rearrange("(r p) c -> p r c", p=PT))
                sums = small.tile([PT, pgw], F32, name="sums", tag="sums")
                nc.vector.tensor_reduce(sums[:, :], agb.rearrange("p r c -> p c r"),
                                        AXX, ADD)

                for jj in range(grp_tiles):
                    j = g * grp_tiles + jj
                    et, cl = ets[jj], cls[jj]
                    c = 4 * jj
                    # scales: head 1/Z_h ; tail_i cl_i/Z_h/Z_i
                    sc = small.tile([PT, 8], F32, name="sc", tag=f"sc{jj}")
                    nc.vector.tensor_add(sc[:, 0:1], sums[:, c:c + 1], cl[:, 0:1])
                    nc.vector.tensor_add(sc[:, 0:1], sc[:, 0:1], cl[:, 1:2])
                    nc.vector.reciprocal(sc[:, 1:2], sc[:, 0:1])          # 1/Z_h
                    nc.vector.reciprocal(sc[:, 2:3], sums[:, c + 1:c + 2])  # 1/Z_t0
                    nc.vector.reciprocal(sc[:, 3:4], sums[:, c + 2:c + 3])  # 1/Z_t1
                    nc.vector.tensor_mul(sc[:, 4:5], cl[:, 0:1], sc[:, 1:2])
                    nc.vector.tensor_mul(sc[:, 5:6], sc[:, 4:5], sc[:, 2:3])
                    nc.vector.tensor_mul(sc[:, 6:7], cl[:, 1:2], sc[:, 1:2])
                    nc.vector.tensor_mul(sc[:, 7:8], sc[:, 6:7], sc[:, 3:4])

                    # normalize in place (bf16 -> 4x DVE mode)
                    nc.vector.tensor_scalar_mul(et[:, 0:HEAD], et[:, 0:HEAD],
                                                sc[:, 1:2])
                    nc.vector.tensor_scalar_mul(et[:, HEAD:HEAD + T0],
                                                et[:, HEAD:HEAD + T0], sc[:, 5:6])
                    nc.vector.tensor_scalar_mul(et[:, HEAD + T0:OUT_COLS],
                                                et[:, HEAD + T0:OUT_COLS],
                                                sc[:, 7:8])

                    # output shard rows; bf16 -> f32 cast in the DMA (SWDGE)
                    eng = nc.sync if et_f32 else nc.gpsimd
                    step = OUT_COLS // split_out
                    for h in range(split_out):
                        c0, c1 = h * step, (h + 1) * step if h < split_out - 1 else OUT_COLS
                        eng.dma_start(out=out_d[ts(j, PT), c0:c1],
                                      in_=et[:, c0:c1])

    nc.compile()
    return nc


_NC_CACHE: dict = {}


def _get_nc(repeats: int = 1):
    if repeats not in _NC_CACHE:
        _NC_CACHE[repeats] = build_nc(repeats)
    return _NC_CACHE[repeats]


def make_in_maps(inputs: dict) -> list[dict]:
    bf16 = ml_dtypes.bfloat16
    x = np.asarray(inputs["x"], dtype=np.float32)
    head_weight = np.asarray(inputs["head_weight"], dtype=np.float32)
    tp0 = np.asarray(inputs["tail_proj_0"], dtype=np.float32)
    tw0 = np.asarray(inputs["tail_w_0"], dtype=np.float32)
    tp1 = np.asarray(inputs["tail_proj_1"], dtype=np.float32)
    tw1 = np.asarray(inputs["tail_w_1"], dtype=np.float32)

    xt = np.ascontiguousarray(x.reshape(TOK, H).T).astype(bf16)   # [512, 2048]
    cluster = head_weight[:, 8 * HEAD:8 * HEAD + 2]
    tp0_b = np.ascontiguousarray(tp0).astype(bf16)
    tp1_b = np.ascontiguousarray(tp1).astype(bf16)
    in_maps = []
    for c in range(NCORES):
        hw_c = np.concatenate(
            [head_weight[:, c * HEAD:(c + 1) * HEAD], cluster], axis=1
        ).astype(bf16)
        in_maps.append({
            "xt": xt,
            "hw": np.ascontiguousarray(hw_c),
            "tp0": tp0_b,
            "tw0": np.ascontiguousarray(tw0[:, c * T0:(c + 1) * T0]).astype(bf16),
            "tp1": tp1_b,
            "tw1": np.ascontiguousarray(tw1[:, c * T1:(c + 1) * T1]).astype(bf16),
        })
    return in_maps


def assemble(outs: list[np.ndarray]) -> np.ndarray:
    head = np.concatenate([o[:, :HEAD] for o in outs], axis=1)
    t0 = np.concatenate([o[:, HEAD:HEAD + T0] for o in outs], axis=1)
    t1 = np.concatenate([o[:, HEAD + T0:OUT_COLS] for o in outs], axis=1)
    return np.concatenate([head, t0, t1], axis=1).reshape(2, 1024, 100000)


def kernel(**inputs) -> np.ndarray:
    in_maps = make_in_maps(inputs)
    nc = _get_nc(1)
    res = run_bass_kernel_spmd(nc, in_maps, core_ids=list(range(NCORES)))
    outs = [np.asarray(res.results[c]["out"], dtype=np.float32)
            for c in range(NCORES)]
    return assemble(outs).astype(np.float32)


if __name__ == "__main__":
    rng = np.random.default_rng(0)
    ins = {
        "x": rng.standard_normal((2, 1024, 512), dtype=np.float32),
        "head_weight": rng.standard_normal((512, 20002), dtype=np.float32) * 0.02,
        "tail_proj_0": rng.standard_normal((512, 128), dtype=np.float32) * 0.02,
        "tail_w_0": rng.standard_normal((128, 30000), dtype=np.float32) * 0.02,
        "tail_proj_1": rng.standard_normal((512, 32), dtype=np.float32) * 0.02,
        "tail_w_1": rng.standard_normal((32, 50000), dtype=np.float32) * 0.02,
    }
    out = kernel(**ins)
    print(out.shape, out.dtype, out.sum())


# revision 36
# speedup vs baseline: 1.6781x; 1.6781x over previous
"""Adaptive-input softmax (AdaptiveLogSoftmaxWithLoss 'softmax' mode) on 8 TRN2 NeuronCores.

Problem: x [2,1024,512] f32 -> out [2,1024,100000] f32.
  head softmax over 20002 logits (20000 head tokens + 2 tail-cluster logits),
  tail_i softmax over its vocab, scaled by its cluster probability.

Strategy (vocab-parallel over 8 cores):
  Each core owns 1/8 of each softmax group: 2500 head cols + 3750 tail0 cols +
  6250 tail1 cols = a [2048 tokens, 12500] f32 output shard (102 MB — the
  kernel is memory-bound on this write; per-core HBM ~358 GB/s -> ~290 us).
  Per 128-token tile: matmul logits (bf16 inputs, f32 PSUM, [128,2048] PSUM
  tiles), exp on ScalarE into a bf16 SBUF tile (accumulating per-group partial
  sums), AllGather the per-group partial sums across cores every 2 tiles
  (tiny, ~5 us, overlapped) and reduce locally, normalize on VectorE (4x bf16
  mode), DMA out with bf16->f32 cast (SWDGE). The 2 cluster logits are
  computed redundantly on every core (identical values) and added to the
  gathered head-slice sums locally.

Host side: shard/transpose/cast inputs (bf16), reassemble output shards.
"""

import numpy as np
import ml_dtypes
from contextlib import ExitStack

import concourse.bass as bass
import concourse.mybir as mybir
import concourse.tile as tile
from concourse import bacc
from concourse.bass import ts
from concourse.bass_utils import run_bass_kernel_spmd

NCORES = 8
H = 512
TOK = 2048           # 2*1024 tokens
PT = 128             # tokens per tile (partition dim)
NTILE = TOK // PT    # 16
HEAD = 2500          # head vocab shard per core (20000/8)
T0 = 3750            # tail0 shard (30000/8)
T1 = 6250            # tail1 shard (50000/8)
OUT_COLS = HEAD + T0 + T1   # 12500
P0 = 128             # tail0 projection dim
P1 = 32              # tail1 projection dim
BF16 = mybir.dt.bfloat16
F32 = mybir.dt.float32

LCOLS = 2502 + T0 + T1      # 12502 logical logit cols (head+cl | t0 | t1)
PSW = 2048                  # psum tile width (4 banks)
NPT = (LCOLS + PSW - 1) // PSW   # 7 psum tiles per token tile

# logical col segments: which weight/contract path computes a col
SEG_A = (0, 2502)           # head + cluster, K=512 via xT
SEG_B = (2502, 2502 + T0)   # tail0, K=128 via p0T
SEG_C = (2502 + T0, LCOLS)  # tail1, K=32  via p1T
# groups for softmax sums / output placement
G_HEAD = (0, 2500)
G_CL = (2500, 2502)
G_T0 = SEG_B
G_T1 = SEG_C

EXP = mybir.ActivationFunctionType.Exp
ADD = mybir.AluOpType.add
AXX = mybir.AxisListType.X


def _spans(psw: int = PSW):
    """Per psum-tile matmul spans (bank x segment) and act spans (group)."""
    tiles = []
    slot = {"head": [], "t0": [], "t1": []}
    nslots = 0
    npt = (LCOLS + psw - 1) // psw
    for t in range(npt):
        t0c, t1c = t * psw, min((t + 1) * psw, LCOLS)
        mms = []
        for b in range(t0c, t1c, 512):
            be = min(b + 512, t1c)
            for (seg, (s0, s1)) in (("A", SEG_A), ("B", SEG_B), ("C", SEG_C)):
                g0, g1 = max(b, s0), min(be, s1)
                if g0 < g1:
                    mms.append((seg, g0, g1))
        acts = []
        for (grp, (s0, s1)) in (("head", G_HEAD), ("cl", G_CL),
                                ("t0", G_T0), ("t1", G_T1)):
            g0, g1 = max(t0c, s0), min(t1c, s1)
            if g0 < g1:
                if grp == "cl":
                    acts.append((grp, g0, g1, None))
                else:
                    acts.append((grp, g0, g1, nslots))
                    slot[grp].append(nslots)
                    nslots += 1
        tiles.append((t0c, mms, acts))
    return tiles, slot, nslots


PT_SPANS, SLOTS, NACC = _spans()
assert NACC == 9, NACC
# slot ranges must be contiguous per group for the reduce
assert SLOTS["head"] == [0, 1] and SLOTS["t0"] == [2, 3, 4] \
    and SLOTS["t1"] == [5, 6, 7, 8]


def _slot_ranges(slots):
    r = {}
    for g, sl in slots.items():
        assert sl == list(range(sl[0], sl[-1] + 1))
        r[g] = (sl[0], sl[-1] + 1)
    return r

GRP_TILES = 2               # token tiles batched per AllGather (default)


HALF = HEAD + T0            # 6250 = half-tile boundary (head+t0 | t1)
assert HALF == OUT_COLS - HALF


def build_nc(repeats: int = 1, et_bufs: int = 5, et_f32: bool = False,
             grp_tiles: int = GRP_TILES, split_out: int = 1,
             fake_cc: int = 0, psw: int = PSW, ps_bufs: int = 2,
             dma_only: int = 0, halves: int = 0, use_ar: int = 0,
             protect: int = 0) -> bass.Bass:
    pt_spans, slots, nacc = _spans(psw)
    sr = _slot_ranges(slots)
    nc = bacc.Bacc("TRN2", target_bir_lowering=False, debug=False,
                   num_devices=NCORES)
    xt_d = nc.declare_dram_parameter("xt", [H, TOK], BF16, isOutput=False)
    hw_d = nc.declare_dram_parameter("hw", [H, HEAD + 2], BF16, isOutput=False)
    tp0_d = nc.declare_dram_parameter("tp0", [H, P0], BF16, isOutput=False)
    tw0_d = nc.declare_dram_parameter("tw0", [P0, T0], BF16, isOutput=False)
    tp1_d = nc.declare_dram_parameter("tp1", [H, P1], BF16, isOutput=False)
    tw1_d = nc.declare_dram_parameter("tw1", [P1, T1], BF16, isOutput=False)
    out_d = nc.declare_dram_parameter("out", [TOK, OUT_COLS], F32, isOutput=True)

    et_dt = F32 if et_f32 else BF16

    with tile.TileContext(nc) as tc, ExitStack() as ctx:
        singles = ctx.enter_context(tc.tile_pool(name="singles", bufs=1))
        psum = ctx.enter_context(tc.tile_pool(name="psum", bufs=ps_bufs, space="PSUM"))
        etp = ctx.enter_context(tc.tile_pool(name="etp", bufs=et_bufs))
        small = ctx.enter_context(tc.tile_pool(name="small", bufs=4))
        dram = ctx.enter_context(tc.tile_pool(name="dram", bufs=4, space="DRAM"))

        # ---- stage weights + xT in SBUF (bf16) ----
        xt_sb = singles.tile([PT, 4, TOK], BF16, name="xt_sb")
        hw_sb = singles.tile([PT, 4, HEAD + 2], BF16, name="hw_sb")
        tp0_sb = singles.tile([PT, 4, P0], BF16, name="tp0_sb")
        tp1_sb = singles.tile([PT, 4, P1], BF16, name="tp1_sb")
        tw0_sb = singles.tile([P0, T0], BF16, name="tw0_sb")
        tw1_sb = singles.tile([P1, T1], BF16, name="tw1_sb")
        for s in range(4):
            nc.sync.dma_start(out=xt_sb[:, s, :], in_=xt_d[ts(s, PT), :])
            nc.sync.dma_start(out=hw_sb[:, s, :], in_=hw_d[ts(s, PT), :])
            nc.sync.dma_start(out=tp0_sb[:, s, :], in_=tp0_d[ts(s, PT), :])
            nc.sync.dma_start(out=tp1_sb[:, s, :], in_=tp1_d[ts(s, PT), :])
        nc.sync.dma_start(out=tw0_sb[:, :], in_=tw0_d[:, :])
        nc.sync.dma_start(out=tw1_sb[:, :], in_=tw1_d[:, :])

        # ---- low-rank projections, transposed: p0T [128, 2048], p1T [32, 2048]
        p0t_sb = singles.tile([P0, TOK], BF16, name="p0t_sb")
        p1t_sb = singles.tile([P1, TOK], BF16, name="p1t_sb")
        for c0 in range(0, TOK, psw):
            w = min(psw, TOK - c0)
            ps0 = psum.tile([PT, psw], F32, name="ps0", tag="ps")
            ps1 = psum.tile([PT, psw], F32, name="ps1", tag="ps")
            for nb in range(w // 512):
                for k in range(4):
                    nc.tensor.matmul(ps0[:, ts(nb, 512)], tp0_sb[:, k, :],
                                     xt_sb[:, k, c0 + nb * 512:c0 + (nb + 1) * 512],
                                     start=(k == 0), stop=(k == 3))
                for k in range(4):
                    nc.tensor.matmul(ps1[:P1, ts(nb, 512)], tp1_sb[:, k, :],
                                     xt_sb[:, k, c0 + nb * 512:c0 + (nb + 1) * 512],
                                     start=(k == 0), stop=(k == 3))
            nc.vector.tensor_copy(p0t_sb[:, c0:c0 + w], ps0[:, :w])
            nc.vector.tensor_copy(p1t_sb[:, c0:c0 + w], ps1[:P1, :w])

        if dma_only:
            # timing probe: only the output DMAs, sourced from the (already
            # loaded) weight tiles. Output is garbage; never use for results.
            xt_flat = xt_sb.rearrange("p a b -> p (a b)")
            hw_flat = hw_sb.rearrange("p a b -> p (a b)")
            for r in range(repeats):
                for j in range(NTILE):
                    if et_f32:
                        xf = xt_flat.bitcast(F32)
                        hf = hw_flat.bitcast(F32)
                        for h in (0, HALF):
                            nc.sync.dma_start(
                                out=out_d[ts(j, PT), h:h + 4096], in_=xf[:, :4096])
                            nc.sync.dma_start(
                                out=out_d[ts(j, PT), h + 4096:h + HALF],
                                in_=hf[:, :HALF - 4096])
                    else:
                        for h in (0, HALF):
                            nc.gpsimd.dma_start(
                                out=out_d[ts(j, PT), h:h + HALF],
                                in_=xt_flat[:, :HALF])
                    if protect:
                        # read back a sliver of every written region so no
                        # write is provably dead (guards against any
                        # dead-store elimination skewing the repeat timing)
                        rb = small.tile([PT, 8], F32, name="rb", tag="rb")
                        nc.sync.dma_start(out=rb[:, 0:4],
                                          in_=out_d[ts(j, PT), 0:4])
                        nc.sync.dma_start(out=rb[:, 4:8],
                                          in_=out_d[ts(j, PT), HALF:HALF + 4])
            repeats = 0  # skip the real main loop below

        # ---- main loop ----
        ngrp = NTILE // grp_tiles
        pgw = 4 * grp_tiles
        for r in range(repeats):
            for g in range(ngrp):
                ets, cls = [], []
                pg = small.tile([PT, pgw], F32, name="pg", tag="pg")
                for jj in range(grp_tiles):
                    j = g * grp_tiles + jj
                    if halves:
                        eta = etp.tile([PT, HALF], et_dt, name="eta", tag="et")
                        etb = etp.tile([PT, HALF], et_dt, name="etb", tag="et")
                    else:
                        et = etp.tile([PT, OUT_COLS], et_dt, name="et", tag="et")
                        eta = etb = None
                    cl = small.tile([PT, 2], F32, name="cl", tag=f"cl{jj}")
                    sacc = small.tile([PT, nacc], F32, name="sacc", tag=f"sacc{jj}")
                    ets.append((None if halves else et, eta, etb))
                    cls.append(cl)
                    for (base, mms, acts) in pt_spans:
                        pt = psum.tile([PT, psw], F32, name="pt", tag="ps")
                        for (seg, g0, g1) in mms:
                            o = g0 - base
                            n = g1 - g0
                            if seg == "A":
                                for k in range(4):
                                    nc.tensor.matmul(
                                        pt[:, o:o + n], xt_sb[:, k, ts(j, PT)],
                                        hw_sb[:, k, g0:g1],
                                        start=(k == 0), stop=(k == 3))
                            elif seg == "B":
                                nc.tensor.matmul(
                                    pt[:, o:o + n], p0t_sb[:, ts(j, PT)],
                                    tw0_sb[:, g0 - SEG_B[0]:g1 - SEG_B[0]])
                            else:
                                nc.tensor.matmul(
                                    pt[:, o:o + n], p1t_sb[:, ts(j, PT)],
                                    tw1_sb[:, g0 - SEG_C[0]:g1 - SEG_C[0]])
                        for (grp, g0, g1, sl) in acts:
                            o = g0 - base
                            n = g1 - g0
                            if grp == "cl":
                                nc.scalar.activation(cl[:, :], pt[:, o:o + n], EXP)
                                continue
                            # output col: head at g0, t0/t1 shifted by the 2 cl cols
                            oc = g0 if grp == "head" else g0 - 2
                            if halves:
                                dst = eta if oc < HALF else etb
                                hc = oc if oc < HALF else oc - HALF
                                dst_ap = dst[:, hc:hc + n]
                            else:
                                dst_ap = et[:, oc:oc + n]
                            nc.scalar.activation(dst_ap, pt[:, o:o + n],
                                                 EXP, accum_out=sacc[:, sl:sl + 1])
                    # per-group partial sums -> pg cols [4*jj .. 4*jj+3)
                    c = 4 * jj
                    for gi, grp in enumerate(("head", "t0", "t1")):
                        s0, s1 = sr[grp]
                        nc.vector.tensor_reduce(pg[:, c + gi:c + gi + 1],
                                                sacc[:, s0:s1], AXX, ADD)
                    nc.vector.tensor_reduce(pg[:, c + 3:c + 4],
                                            sacc[:, nacc - 1:nacc], AXX, ADD)

                # cross-core reduction of the partial softmax denominators
                rg = ([[c] for c in range(NCORES)] if fake_cc
                      else [list(range(NCORES))])
                ccin = dram.tile([PT, pgw], F32, name="ccin", tag="ccin")
                nc.sync.dma_start(out=ccin[:, :], in_=pg[:, :])
                sums = small.tile([PT, pgw], F32, name="sums", tag="sums")
                if use_ar:
                    ccr = dram.tile([PT, pgw], F32, name="ccr", tag="ccr",
                                    addr_space="Shared")
                    nc.gpsimd.collective_compute(
                        "AllReduce", ADD, replica_groups=rg,
                        ins=[ccin[:, :].opt()], outs=[ccr[:, :].opt()])
                    nc.sync.dma_start(out=sums[:, :], in_=ccr[:, :])
                else:
                    ccout = dram.tile([NCORES * PT, pgw], F32, name="ccout",
                                      tag="ccout", addr_space="Shared")
                    nc.gpsimd.collective_compute(
                        "AllGather", mybir.AluOpType.bypass,
                        replica_groups=rg,
                        ins=[ccin[:, :].opt()],
                        outs=[(ccout[:PT, :] if fake_cc else ccout[:, :]).opt()])
                    agb = small.tile([PT, NCORES, pgw], F32, name="agb", tag="agb")
                    nc.sync.dma_start(
                        out=agb[:, :, :],
                        in_=ccout.rearrange("(r p) c -> p r c", p=PT))
                    nc.vector.tensor_reduce(sums[:, :],
                                            agb.rearrange("p r c -> p c r"),
                                            AXX, ADD)

                for jj in range(grp_tiles):
                    j = g * grp_tiles + jj
                    (et, eta, etb), cl = ets[jj], cls[jj]
                    c = 4 * jj
                    # scales: head 1/Z_h ; tail_i cl_i/Z_h/Z_i
                    sc = small.tile([PT, 8], F32, name="sc", tag=f"sc{jj}")
                    nc.vector.tensor_add(sc[:, 0:1], sums[:, c:c + 1], cl[:, 0:1])
                    nc.vector.tensor_add(sc[:, 0:1], sc[:, 0:1], cl[:, 1:2])
                    nc.vector.reciprocal(sc[:, 1:2], sc[:, 0:1])          # 1/Z_h
                    nc.vector.reciprocal(sc[:, 2:3], sums[:, c + 1:c + 2])  # 1/Z_t0
                    nc.vector.reciprocal(sc[:, 3:4], sums[:, c + 2:c + 3])  # 1/Z_t1
                    nc.vector.tensor_mul(sc[:, 4:5], cl[:, 0:1], sc[:, 1:2])
                    nc.vector.tensor_mul(sc[:, 5:6], sc[:, 4:5], sc[:, 2:3])
                    nc.vector.tensor_mul(sc[:, 6:7], cl[:, 1:2], sc[:, 1:2])
                    nc.vector.tensor_mul(sc[:, 7:8], sc[:, 6:7], sc[:, 3:4])

                    # normalize in place (bf16 -> 4x DVE mode), then DMA out
                    # (bf16 -> f32 cast happens in the SWDGE DMA)
                    eng = nc.sync if et_f32 else nc.gpsimd
                    if halves:
                        nc.vector.tensor_scalar_mul(eta[:, 0:HEAD], eta[:, 0:HEAD],
                                                    sc[:, 1:2])
                        nc.vector.tensor_scalar_mul(eta[:, HEAD:HALF],
                                                    eta[:, HEAD:HALF], sc[:, 5:6])
                        nc.vector.tensor_scalar_mul(etb[:, :], etb[:, :], sc[:, 7:8])
                        eng.dma_start(out=out_d[ts(j, PT), 0:HALF], in_=eta[:, :])
                        eng.dma_start(out=out_d[ts(j, PT), HALF:OUT_COLS],
                                      in_=etb[:, :])
                    else:
                        nc.vector.tensor_scalar_mul(et[:, 0:HEAD], et[:, 0:HEAD],
                                                    sc[:, 1:2])
                        nc.vector.tensor_scalar_mul(et[:, HEAD:HALF],
                                                    et[:, HEAD:HALF], sc[:, 5:6])
                        nc.vector.tensor_scalar_mul(et[:, HALF:OUT_COLS],
                                                    et[:, HALF:OUT_COLS],
                                                    sc[:, 7:8])
                        eng.dma_start(out=out_d[ts(j, PT), :], in_=et[:, :])

    nc.compile()
    return nc


_NC_CACHE: dict = {}


def _get_nc(repeats: int = 1):
    if repeats not in _NC_CACHE:
        _NC_CACHE[repeats] = build_nc(repeats)
    return _NC_CACHE[repeats]


def make_in_maps(inputs: dict) -> list[dict]:
    bf16 = ml_dtypes.bfloat16
    x = np.asarray(inputs["x"], dtype=np.float32)
    head_weight = np.asarray(inputs["head_weight"], dtype=np.float32)
    tp0 = np.asarray(inputs["tail_proj_0"], dtype=np.float32)
    tw0 = np.asarray(inputs["tail_w_0"], dtype=np.float32)
    tp1 = np.asarray(inputs["tail_proj_1"], dtype=np.float32)
    tw1 = np.asarray(inputs["tail_w_1"], dtype=np.float32)

    xt = np.ascontiguousarray(x.reshape(TOK, H).T).astype(bf16)   # [512, 2048]
    cluster = head_weight[:, 8 * HEAD:8 * HEAD + 2]
    tp0_b = np.ascontiguousarray(tp0).astype(bf16)
    tp1_b = np.ascontiguousarray(tp1).astype(bf16)
    in_maps = []
    for c in range(NCORES):
        hw_c = np.concatenate(
            [head_weight[:, c * HEAD:(c + 1) * HEAD], cluster], axis=1
        ).astype(bf16)
        in_maps.append({
            "xt": xt,
            "hw": np.ascontiguousarray(hw_c),
            "tp0": tp0_b,
            "tw0": np.ascontiguousarray(tw0[:, c * T0:(c + 1) * T0]).astype(bf16),
            "tp1": tp1_b,
            "tw1": np.ascontiguousarray(tw1[:, c * T1:(c + 1) * T1]).astype(bf16),
        })
    return in_maps


def assemble(outs: list[np.ndarray]) -> np.ndarray:
    head = np.concatenate([o[:, :HEAD] for o in outs], axis=1)
    t0 = np.concatenate([o[:, HEAD:HEAD + T0] for o in outs], axis=1)
    t1 = np.concatenate([o[:, HEAD + T0:OUT_COLS] for o in outs], axis=1)
    return np.concatenate([head, t0, t1], axis=1).reshape(2, 1024, 100000)


def kernel(**inputs) -> np.ndarray:
    in_maps = make_in_maps(inputs)
    nc = _get_nc(1)
    res = run_bass_kernel_spmd(nc, in_maps, core_ids=list(range(NCORES)))
    outs = [np.asarray(res.results[c]["out"], dtype=np.float32)
            for c in range(NCORES)]
    return assemble(outs).astype(np.float32)


if __name__ == "__main__":
    rng = np.random.default_rng(0)
    ins = {
        "x": rng.standard_normal((2, 1024, 512), dtype=np.float32),
        "head_weight": rng.standard_normal((512, 20002), dtype=np.float32) * 0.02,
        "tail_proj_0": rng.standard_normal((512, 128), dtype=np.float32) * 0.02,
        "tail_w_0": rng.standard_normal((128, 30000), dtype=np.float32) * 0.02,
        "tail_proj_1": rng.standard_normal((512, 32), dtype=np.float32) * 0.02,
        "tail_w_1": rng.standard_normal((32, 50000), dtype=np.float32) * 0.02,
    }
    out = kernel(**ins)
    print(out.shape, out.dtype, out.sum())
